# revision 43
# baseline (speedup 1.0000x reference)
# CQAttention (QANet context-query attention) Trainium2 kernel, v2.
#
# Full-input contract: kernel(**inputs) takes the complete unsharded arrays
# and returns the full [B, 4D, Lc] output. Internally shards batch across the
# 8 NeuronCores (8 batches per core), runs one SPMD Bass program, and
# concatenates the per-core results.
#
# Math (per batch b, with Ct = C[b].T, Qt = Q[b].T):
#   S  = Ct@w4C + (Qt@w4Q).T + (Ct*w4mlu)@Qt.T + bias      [Lc, Lq]
#   S1 = softmax_q(S), S2 = softmax_c(S)   (masks all-ones, bias cancels)
#   A  = S1@Qt ; Bm = S1@(S2.T@Ct)         (S12 reassociated away)
#   out[b] = [C; A.T; C*A.T; C*Bm.T]       [4D, Lc]
#
# Implementation identities:
#   - E = exp(s2+s1-SHIFT): s0 and the shift cancel in the row softmax
#     (S1 = E/r1); for the column softmax es0=exp(s0) is folded into Ct
#     (Ct_es = Ct*es0) and the r2 reduction (rhs = es0).
#   - A^T_raw = Qt^T@Et8 with Qt = Qhi8+Qlo8 (double-fp8 split: keeps Q at
#     ~bf16 accuracy) and Bm^T_raw = Tq8^T@Et8, all as fp8 DoubleRow matmuls
#     (256-deep contraction steps at 0.5 cyc/row); normalized by an r1inv
#     row broadcast (r1bc) built with selector matmuls.  r1 is summed from
#     Et8 itself so fp8 quantization partly cancels in the normalization.
#   - T = S2^T@Ct_es accumulated directly in [q,d] layout (no transpose).
#   - E^T comes from XBAR DMA transposes (SBUF->SBUF bf16): zero compute
#     engine time.  E/Et/Et8 are half-tiles so the transpose/cast/matmul
#     chain pipelines.
#   - block0 of the output is C itself and never touches the device.
#
# Host-side prep is limited to dtype casts and layout transposes of the
# inputs; all FLOPs stay on device.

import numpy as np
import ml_dtypes

B, D, LC, LQ = 64, 128, 1024, 512
N_CORES = 8
BPC = B // N_CORES  # batches per core
NCH_C = LC // 128   # 8 c-chunks
NCH_Q = LQ // 128   # 4 q-chunks
SHIFT = 6.4         # exp shift; hi-cast clamps at 239 (TRN2 fp8e4 infs at 256,
                    # max normal 240) and the Elo residual absorbs the clamp.

_compiled = {}


def build_nc(bpc: int):
    import concourse.bass as bass
    import concourse.mybir as mybir
    import concourse.tile as tile
    from concourse import bacc
    from concourse.masks import make_identity

    f32 = mybir.dt.float32
    bf16 = mybir.dt.bfloat16
    fp8 = mybir.dt.float8e4
    AF = mybir.ActivationFunctionType
    OP = mybir.AluOpType
    PM = mybir.MatmulPerfMode

    nc = bacc.Bacc()

    Cbf_d = nc.declare_dram_parameter("Cbf", (bpc, D, LC), bf16, isOutput=False)
    Ctbf_d = nc.declare_dram_parameter("Ctbf", (bpc, 128, NCH_C * D), bf16, isOutput=False)
    # Qpack: per-partition 2048 bytes = Qbf [512 bf16] | Qhi8 [512 u8] | Qlo8 [512 u8]
    Qpack_d = nc.declare_dram_parameter("Qpack", (bpc, 128, 2048), mybir.dt.uint8, isOutput=False)
    w4C_d = nc.declare_dram_parameter("w4C", (D, 1), f32, isOutput=False)
    w4Q_d = nc.declare_dram_parameter("w4Q", (D, 1), f32, isOutput=False)
    w4mlu_d = nc.declare_dram_parameter("w4mlu", (1, 1, D), f32, isOutput=False)
    sel_d = nc.declare_dram_parameter("sel", (8, 8 * 128), bf16, isOutput=False)
    out1_d = nc.declare_dram_parameter("out1", (bpc, D, LC), bf16, isOutput=True)
    out23_d = nc.declare_dram_parameter("out23", (bpc, 2 * D, LC), bf16, isOutput=True)

    with tile.TileContext(nc) as tc:
        with (
            tc.tile_pool(name="const", bufs=1) as constp,
            tc.tile_pool(name="io", bufs=6) as iop,
            tc.tile_pool(name="work", bufs=4) as workp,
            tc.tile_pool(name="stage", bufs=4) as stagep,
            tc.tile_pool(name="psS", bufs=2, space="PSUM") as psS,    # 2x2 banks
            tc.tile_pool(name="psSm", bufs=1, space="PSUM") as psSm,  # 1 bank
            tc.tile_pool(name="psAB", bufs=3, space="PSUM") as psAB,  # 3 banks
        ):
            # ---- constants (once) ----
            w4mlu_raw = constp.tile([D, 1], f32, tag="w4mlu_r")
            w4Q_raw = constp.tile([D, 1], f32, tag="w4Q_r")
            w4C_raw = constp.tile([D, 1], f32, tag="w4C_r")
            nc.sync.dma_start(out=w4mlu_raw[:], in_=w4mlu_d.rearrange("a b d -> d (a b)"))
            nc.sync.dma_start(out=w4Q_raw[:], in_=w4Q_d[:])
            nc.sync.dma_start(out=w4C_raw[:], in_=w4C_d[:])
            sel_sb = constp.tile([8, 8, 128], bf16, tag="sel")
            nc.sync.dma_start(out=sel_sb[:].rearrange("p a b -> p (a b)"), in_=sel_d[:])
            # funnel through DVE so consumers depend on one engine
            w4mlu_sb = constp.tile([D, 1], f32, tag="w4mlu")
            w4Q_sb = constp.tile([D, 1], f32, tag="w4Qv")
            w4Cb_sb = constp.tile([D, 1], bf16, tag="w4Cb")
            nc.vector.tensor_copy(out=w4mlu_sb[:], in_=w4mlu_raw[:])
            nc.vector.tensor_copy(out=w4Q_sb[:], in_=w4Q_raw[:])
            nc.vector.tensor_copy(out=w4Cb_sb[:], in_=w4C_raw[:])
            ident_sb = constp.tile([128, 128], bf16, tag="ident")
            make_identity(nc, ident_sb[:])
            ones8_sb = constp.tile([128, 1], fp8, tag="ones8")
            nc.gpsimd.memset(ones8_sb[:], 1.0)
            shift_sb = constp.tile([128, 1], f32, tag="shift")
            nc.gpsimd.memset(shift_sb[:], -SHIFT)
            zero_sb = constp.tile([128, 1], f32, tag="zero")
            nc.gpsimd.memset(zero_sb[:], 0.0)

            def stage1(b):
                # loads + input prep for batch b
                C_sb = iop.tile([D, LC], bf16, tag="C")
                Ct_sb = iop.tile([128, NCH_C, D], bf16, tag="Ct")
                Qpack = iop.tile([128, 2048], mybir.dt.uint8, tag="Qp")
                nc.sync.dma_start(out=C_sb[:], in_=Cbf_d[b])
                nc.sync.dma_start(out=Ct_sb[:].rearrange("p a b -> p (a b)"), in_=Ctbf_d[b])
                nc.scalar.dma_start(out=Qpack[:], in_=Qpack_d[b])
                Q_sb = Qpack[:, 0:1024].bitcast(bf16)           # [128, 512]
                Qhi8 = Qpack[:, 1024:1536].bitcast(fp8)         # [128, 512] = [qm, (j d)]
                Qlo8 = Qpack[:, 1536:2048].bitcast(fp8)

                # Cw' = C*w4mlu + w4Q  (emits s2+s1 in one matmul)
                Cwp = workp.tile([D, LC], bf16, tag="Cwp")
                nc.vector.tensor_scalar(
                    out=Cwp[:], in0=C_sb[:],
                    scalar1=w4mlu_sb[:], scalar2=w4Q_sb[:],
                    op0=OP.mult, op1=OP.add,
                )
                # s0p[c] = sum_d C[d,c] w4C[d]; es0 = exp(s0)
                s0p_ps = psAB.tile([128, NCH_C], f32, tag="AB")
                for cj in range(NCH_C):
                    nc.tensor.matmul(
                        out=s0p_ps[:, cj:cj + 1],
                        lhsT=C_sb[:, cj * 128:(cj + 1) * 128],
                        rhs=w4Cb_sb[:],
                        start=True, stop=True,
                    )
                es0 = workp.tile([128, NCH_C], f32, tag="es0")
                nc.scalar.activation(out=es0[:], in_=s0p_ps[:], func=AF.Exp,
                                     bias=zero_sb[:], scale=1.0)
                es0b = workp.tile([128, NCH_C], bf16, tag="es0b")
                nc.vector.tensor_copy(out=es0b[:], in_=es0[:])
                # Ct_es = Ct * es0[c]  (per-partition scale per cj slice)
                Ct_es = workp.tile([128, NCH_C, D], bf16, tag="Ct_es")
                for cj in range(NCH_C):
                    nc.vector.tensor_scalar_mul(
                        out=Ct_es[:, cj, :], in0=Ct_sb[:, cj, :],
                        scalar1=es0[:, cj:cj + 1],
                    )
                return dict(C_sb=C_sb, Q_sb=Q_sb, Qhi8=Qhi8, Qlo8=Qlo8,
                            Cwp=Cwp, es0b=es0b, Ct_es=Ct_es)

            def stageB(b, st):
                Q_sb, Cwp = st["Q_sb"], st["Cwp"]
                # ---- scores + exp: E = exp(s2+s1-SHIFT), bf16 [cm, cjl, q]
                # ---- per half (cj = 4*hh + cjl), then transpose + fp8 cast
                Eh = []     # [128, 4, 512] bf16 per half
                Et8q = []   # [128, 4(j), 2(cjl), 128] fp8 per quarter (cj pair)
                Elo8q = []  # residuals: Et - Et8 (absorbs clamp + flush floor)
                for hh in range(2):
                    E = workp.tile([128, 4, LQ], bf16, tag=f"E{hh}")
                    for t in range(2):
                        S_ps = psS.tile([128, 2, LQ], f32, tag="S")
                        for k in range(2):
                            cj = 4 * hh + 2 * t + k
                            nc.tensor.matmul(
                                out=S_ps[:, k, :],
                                lhsT=Cwp[:, cj * 128:(cj + 1) * 128],
                                rhs=Q_sb[:],
                                start=True, stop=True,
                            )
                        nc.scalar.activation(
                            out=E[:, 2 * t:2 * t + 2, :].rearrange("p a b -> p (a b)"),
                            in_=S_ps[:].rearrange("p a b -> p (a b)"),
                            func=AF.Exp, bias=shift_sb[:], scale=1.0,
                        )
                    Et = workp.tile([128, 4, NCH_Q, 128], bf16, tag=f"Et{hh}")
                    nc.sync.dma_start_transpose(
                        Et[:], E[:].rearrange("p a b -> p (a b)"))
                    for t in range(2):
                        g = 2 * hh + t
                        Et8 = workp.tile([128, NCH_Q, 2, 128], fp8, tag=f"Et8{g}")
                        Elo8 = workp.tile([128, NCH_Q, 2, 128], fp8, tag=f"Elo8{g}")
                        for j in range(NCH_Q):
                            nc.gpsimd.tensor_scalar_min(
                                out=Et8[:, j, :, :],
                                in0=Et[:, 2 * t:2 * t + 2, j, :],
                                scalar1=239.0)
                        for j in range(NCH_Q):
                            nc.gpsimd.tensor_sub(
                                out=Elo8[:, j, :, :],
                                in0=Et[:, 2 * t:2 * t + 2, j, :],
                                in1=Et8[:, j, :, :])
                        Et8q.append(Et8)
                        Elo8q.append(Elo8)
                    Eh.append(E)
                es0b, Ct_es = st["es0b"], st["Ct_es"]

                # ---- r2[q] = sum_c E*es0 ; r1[c] = sum_q Et8 (tiny matmuls) ----
                r2p_ps = psSm.tile([128, NCH_Q], f32, tag="Sm")
                for j in range(NCH_Q):
                    for cj in range(NCH_C):
                        nc.tensor.matmul(
                            out=r2p_ps[:, j:j + 1],
                            lhsT=Eh[cj // 4][:, cj % 4, j * 128:(j + 1) * 128],
                            rhs=es0b[:, cj:cj + 1],
                            start=(cj == 0), stop=(cj == NCH_C - 1),
                        )
                r1p_ps = psSm.tile([128, NCH_C], f32, tag="Sm")
                for cj in range(NCH_C):
                    for j in range(NCH_Q):
                        nc.tensor.matmul(
                            out=r1p_ps[:, cj:cj + 1],
                            lhsT=Et8q[cj // 2][:, j, cj % 2, :],
                            rhs=ones8_sb[:],
                            start=(j == 0), stop=False,
                        )
                    for j in range(NCH_Q):
                        nc.tensor.matmul(
                            out=r1p_ps[:, cj:cj + 1],
                            lhsT=Elo8q[cj // 2][:, j, cj % 2, :],
                            rhs=ones8_sb[:],
                            start=False, stop=(j == NCH_Q - 1),
                        )
                r2inv = workp.tile([128, NCH_Q], f32, tag="r2inv")
                nc.vector.reciprocal(out=r2inv[:], in_=r2p_ps[:])
                r1inv = workp.tile([128, NCH_C], bf16, tag="r1inv")
                with nc.allow_low_precision(reason="r1inv feeds bf16 r1bc"):
                    nc.vector.reciprocal(out=r1inv[:], in_=r1p_ps[:])
                # r1T[cj, cm] = r1inv^T for the broadcast matmuls
                r1T_ps = psSm.tile([8, 128], bf16, tag="Sm")
                nc.tensor.transpose(r1T_ps[:], r1inv[:], ident_sb[:])
                r1T = workp.tile([8, 128], bf16, tag="r1T")
                nc.vector.tensor_copy(out=r1T[:], in_=r1T_ps[:])

                # ---- T[q,d] = sum_c E[c,q] Ct_es[c,d]; Tq8 = T * r2inv ----
                T_ps = psSm.tile([128, NCH_Q, D], f32, tag="Sm")
                for j in range(NCH_Q):
                    for cj in range(NCH_C):
                        nc.tensor.matmul(
                            out=T_ps[:, j, :],
                            lhsT=Eh[cj // 4][:, cj % 4, j * 128:(j + 1) * 128],
                            rhs=Ct_es[:, cj, :],
                            start=(cj == 0), stop=(cj == NCH_C - 1),
                        )
                Tq8 = workp.tile([128, NCH_Q, D], fp8, tag="Tq8")
                Tlo8 = workp.tile([128, NCH_Q, D], fp8, tag="Tlo8")
                for j in range(NCH_Q):
                    nc.vector.tensor_scalar_mul(
                        out=Tq8[:, j, :], in0=T_ps[:, j, :],
                        scalar1=r2inv[:, j:j + 1],
                    )
                for j in range(NCH_Q):
                    nc.vector.scalar_tensor_tensor(
                        out=Tlo8[:, j, :],
                        in0=T_ps[:, j, :], scalar=r2inv[:, j:j + 1],
                        in1=Tq8[:, j, :],
                        op0=OP.mult, op1=OP.subtract)
                return dict(Eh=Eh, Et8q=Et8q, Elo8q=Elo8q, r1T=r1T, Tq8=Tq8, Tlo8=Tlo8)

            def stageC_gen(b, st, stB):
                C_sb, Q_sb = st["C_sb"], st["Q_sb"]
                Qhi8, Qlo8 = st["Qhi8"], st["Qlo8"]
                Et8q, Elo8q = stB["Et8q"], stB["Elo8q"]
                r1T, Tq8, Tlo8 = stB["r1T"], stB["Tq8"], stB["Tlo8"]

                # ---- A^T, Bm^T (fp8 DoubleRow), normalize, outputs ----
                yield
                A_b = stagep.tile([128, LC], bf16, tag="A_b")
                Bm_b = stagep.tile([128, LC], bf16, tag="Bm_b")
                stage = stagep.tile([128, 2, LC], bf16, tag="stage")
                Qhi = Qhi8.rearrange("p (j d) -> p j d", j=NCH_Q)
                Qlo = Qlo8.rearrange("p (j d) -> p j d", j=NCH_Q)
                for h in range(2):
                    # r1bc[d, c-half] = r1inv[c] via selector matmuls
                    r1bc_ps = psAB.tile([128, 512], f32, tag="AB")
                    for cjl in range(4):
                        cj = 4 * h + cjl
                        nc.tensor.matmul(
                            out=r1bc_ps[:, cjl * 128:(cjl + 1) * 128],
                            lhsT=sel_sb[:, cj, :],
                            rhs=r1T[:],
                            start=True, stop=True,
                        )
                    A_ps = psAB.tile([128, 512], f32, tag="AB")
                    Bm_ps = psAB.tile([128, 512], f32, tag="AB")
                    for g in range(2):
                        Et8 = Et8q[2 * h + g]
                        Elo8 = Elo8q[2 * h + g]
                        ops_A = [(Qhi, Et8), (Qlo, Et8), (Qhi, Elo8)]
                        for oi, (lhs, rt) in enumerate(ops_A):
                            for jp in range(2):
                                rhs = rt[:, 2 * jp:2 * jp + 2, :, :].rearrange(
                                    "p j c m -> p j (c m)")
                                nc.tensor.matmul(
                                    out=A_ps[:, g * 256:(g + 1) * 256],
                                    lhsT=lhs[:, 2 * jp:2 * jp + 2, :],
                                    rhs=rhs,
                                    start=(oi == 0 and jp == 0),
                                    stop=(oi == 2 and jp == 1),
                                    perf_mode=PM.DoubleRow,
                                )
                        ops_B = [(Tq8, Et8), (Tlo8, Et8), (Tq8, Elo8)]
                        for oi, (lhs, rt) in enumerate(ops_B):
                            for jp in range(2):
                                rhs = rt[:, 2 * jp:2 * jp + 2, :, :].rearrange(
                                    "p j c m -> p j (c m)")
                                nc.tensor.matmul(
                                    out=Bm_ps[:, g * 256:(g + 1) * 256],
                                    lhsT=lhs[:, 2 * jp:2 * jp + 2, :],
                                    rhs=rhs,
                                    start=(oi == 0 and jp == 0),
                                    stop=(oi == 2 and jp == 1),
                                    perf_mode=PM.DoubleRow,
                                )
                    sl = slice(h * 512, (h + 1) * 512)
                    # HW: only one PSUM operand per DVE op -> r1bc via SBUF
                    r1bcs = stagep.tile([128, 512], bf16, tag=f"r1bcs{h}")
                    if h == 0:
                        nc.scalar.copy(out=r1bcs[:], in_=r1bc_ps[:])
                    else:
                        nc.vector.tensor_copy(out=r1bcs[:], in_=r1bc_ps[:])
                    nc.vector.tensor_mul(out=A_b[:, sl], in0=A_ps[:], in1=r1bcs[:])
                    nc.vector.tensor_mul(out=Bm_b[:, sl], in0=Bm_ps[:], in1=r1bcs[:])
                    nc.vector.tensor_mul(out=stage[:, 0, sl], in0=A_b[:, sl], in1=C_sb[:, sl])
                    if h == 0:
                        nc.vector.tensor_mul(out=stage[:, 1, sl], in0=Bm_b[:, sl], in1=C_sb[:, sl])
                    else:
                        nc.gpsimd.tensor_mul(out=stage[:, 1, sl], in0=Bm_b[:, sl], in1=C_sb[:, sl])

                nc.sync.dma_start(out=out1_d[b], in_=A_b[:])
                nc.scalar.dma_start(
                    out=out23_d[b, 0:D, :].rearrange("(t d) l -> d t l", t=1),
                    in_=stage[:, 0:1, :],
                )
                nc.gpsimd.dma_start(
                    out=out23_d[b, D:2 * D, :].rearrange("(t d) l -> d t l", t=1),
                    in_=stage[:, 1:2, :],
                )

            # software-pipelined emission, depth 2: batch b+1's score/exp/
            # transpose stream is emitted before batch b's Et8-consuming tail
            # so the PE queue never drains waiting for the transpose chain.
            sts = [stage1(0)]
            sBs = [stageB(0, sts[0])]
            sts.append(stage1(1)); sBs.append(stageB(1, sts[1]))
            for b in range(bpc):
                cgen = stageC_gen(b, sts[b], sBs[b])
                next(cgen)  # C1: r1bc selector matmuls
                if b + 2 < bpc:
                    sts.append(stage1(b + 2)); sBs.append(stageB(b + 2, sts[b + 2]))
                for _ in cgen:  # C2: A/Bm matmuls + normalize + stores
                    pass

    nc.compile()
    return nc


def _get_nc(bpc: int):
    if bpc not in _compiled:
        _compiled[bpc] = build_nc(bpc)
    return _compiled[bpc]


def _sel_host():
    sel = np.zeros((8, 8, 128), np.float32)
    for cj in range(8):
        sel[cj, cj, :] = 1.0
    return sel.reshape(8, 8 * 128).astype(ml_dtypes.bfloat16)


def _prep_host(C, Q):
    """dtype/layout-only prep of the inputs (no math beyond the fp8 split)."""
    bf = ml_dtypes.bfloat16
    f8 = ml_dtypes.float8_e4m3fn
    nb = C.shape[0]
    Cbf = C.astype(bf)                                    # [nb, D, LC]
    # Ct[b, cm, cj, d] = C[b, d, cj*128+cm]
    Ct = np.ascontiguousarray(C.transpose(0, 2, 1)).reshape(nb, NCH_C, 128, D)
    Ctbf = np.ascontiguousarray(Ct.transpose(0, 2, 1, 3)).reshape(nb, 128, NCH_C * D).astype(bf)
    # Qt[b, qm, j, d] = Q[b, d, j*128+qm]; double-fp8 split
    Qt = np.ascontiguousarray(Q.transpose(0, 2, 1)).reshape(nb, NCH_Q, 128, D)
    Qt = np.ascontiguousarray(Qt.transpose(0, 2, 1, 3)).reshape(nb, 128, NCH_Q * D)
    Qhi8 = Qt.astype(f8)
    Qlo8 = (Qt.astype(np.float32) - Qhi8.astype(np.float32)).astype(f8)
    Qbf = Q.astype(bf)
    return Cbf, Ctbf, Qbf, Qhi8, Qlo8


def _pack_q(Qbf, Qhi8, Qlo8):
    nb = Qbf.shape[0]
    Qpack = np.empty((nb, 128, 2048), np.uint8)
    # Q_sb view: [128 partitions=d, 512 q] bf16 -> bytes [128, 1024]
    Qpack[:, :, 0:1024] = Qbf.view(np.uint8).reshape(nb, 128, 1024)
    Qpack[:, :, 1024:1536] = Qhi8.view(np.uint8)
    Qpack[:, :, 1536:2048] = Qlo8.view(np.uint8)
    return Qpack


_runner = None


def _build_runner():
    """Cached SPMD runner: builds the sharded jit once, reuses it per call."""
    import jax
    import jax.numpy as jnp
    from jax.sharding import Mesh, PartitionSpec
    from jax.experimental.shard_map import shard_map
    from concourse import bass2jax

    bass2jax.install_neuronx_cc_hook()
    nc = _get_nc(BPC)

    in_names = ["Cbf", "Ctbf", "Qpack", "w4C", "w4Q", "w4mlu", "sel"]
    out_avals = [
        jax.core.ShapedArray((BPC, D, LC), jnp.bfloat16),
        jax.core.ShapedArray((BPC, 2 * D, LC), jnp.bfloat16),
    ]
    all_in_names = in_names + ["out1", "out23"]
    partition_name = nc.partition_id_tensor.name if nc.partition_id_tensor else None
    if partition_name is not None:
        all_in_names.append(partition_name)

    def _body(*args):
        operands = list(args)
        if partition_name is not None:
            operands.append(bass2jax.partition_id_tensor())
        outs = bass2jax._bass_exec_p.bind(
            *operands,
            out_avals=tuple(out_avals),
            in_names=tuple(all_in_names),
            out_names=("out1", "out23"),
            lowering_input_output_aliases=(),
            sim_require_finite=True,
            sim_require_nnan=True,
            nc=nc,
        )
        return tuple(outs)

    devices = jax.devices()[:N_CORES]
    mesh = Mesh(np.asarray(devices), ("core",))
    n_params = len(in_names)
    in_specs = (PartitionSpec("core"),) * (n_params + 2)
    out_specs = (PartitionSpec("core"),) * 2
    sharded = jax.jit(
        shard_map(_body, mesh=mesh, in_specs=in_specs, out_specs=out_specs,
                  check_rep=False),
        donate_argnums=(n_params, n_params + 1), keep_unused=True,
    )
    return sharded


def kernel(C, Q, Cmask=None, Qmask=None, w4C=None, w4Q=None, w4mlu=None, bias=None):
    # Cmask/Qmask are all-ones and bias cancels in both softmaxes -> unused.
    global _runner
    C = np.ascontiguousarray(np.asarray(C, dtype=np.float32))
    Q = np.ascontiguousarray(np.asarray(Q, dtype=np.float32))
    w4C = np.ascontiguousarray(np.asarray(w4C, dtype=np.float32))
    w4Q = np.ascontiguousarray(np.asarray(w4Q, dtype=np.float32))
    w4mlu = np.ascontiguousarray(np.asarray(w4mlu, dtype=np.float32))

    Cbf, Ctbf, Qbf, Qhi8, Qlo8 = _prep_host(C, Q)
    Qpack = _pack_q(Qbf, Qhi8, Qlo8)
    sel = _sel_host()

    if _runner is None:
        _runner = _build_runner()
    # per-core inputs concatenated on axis 0 (per-device BIR shapes)
    w4C_all = np.concatenate([w4C] * N_CORES, axis=0)
    w4Q_all = np.concatenate([w4Q] * N_CORES, axis=0)
    w4mlu_all = np.concatenate([w4mlu] * N_CORES, axis=0)
    sel_all = np.concatenate([sel] * N_CORES, axis=0)
    zeros1 = np.zeros((B, D, LC), ml_dtypes.bfloat16)
    zeros23 = np.zeros((B, 2 * D, LC), ml_dtypes.bfloat16)
    out1, out23 = _runner(Cbf, Ctbf, Qpack,
                          w4C_all, w4Q_all, w4mlu_all, sel_all,
                          zeros1, zeros23)
    out = np.empty((B, 4 * D, LC), np.float32)
    out[:, 0:D] = C
    out[:, D:2 * D] = np.asarray(out1).astype(np.float32)
    out[:, 2 * D:] = np.asarray(out23).astype(np.float32)
    return out
